# revision 26
# baseline (speedup 1.0000x reference)
"""Causal attention kernel for trn2, sharded over 8 NeuronCores.

Problem (B=4, S=2048, E=2048, H=16, D=128), fp32 in/out:
    qkv = x @ w_qkv; q,k,v = split(qkv)
    q,k,v reshaped (B,S,E)->(B,H,S,D) as a RAW view (no transpose), i.e.
    per (b,h): Q_h = rows [h*128,(h+1)*128) of q[b] reinterpreted [S,D].
    o = softmax(QK^T/sqrt(D) + causal(+1/-10000)) @ V, inverse raw view,
    out = o @ w_out.

The raw view maps head h to a contiguous block of 128 sequence rows, so
the computation splits into B*H = 64 independent tasks; core c gets 8
tasks = rows [c*1024,(c+1)*1024) of x.reshape(B*S, E).  No collectives.

All matmuls run in bf16 (inputs converted, x pre-transposed, and the
weights re-laid-out chunk-major host-side so every DMA chunk is DRAM-
contiguous); accumulation is fp32 in PSUM.  Attention computes
S^T = K Q^T per (q-chunk 512, k-tile 128) with causal diagonal tiles cut
to their live q-range, exp batched per [128,<=1024] PSUM generation, a
softmax denominator via ones-stationary matmuls (partition-dim sum
broadcast), and a software-pipelined generation loop (QK lookahead 2).
Each task's output-projection chains and the next tasks' V-transposes
are deferred into a fill queue and injected at later tasks' q-chunk
starts, filling the exp-latency bubbles on the PE; leftovers drain into
the next group's QKV stream.
"""

import numpy as np

B, S, E = 4, 2048, 2048
H, D, P = 16, 128, 128
NCORES = 8
ROWS = B * S // NCORES   # 1024 rows per core = 8 tasks of 128 rows
NGRP = 2                 # task groups per core
NTT = 4                  # tasks per group
SCALE = float(1.0 / np.sqrt(D))
NEG = -1.0e9  # pre-scale additive mask; exp underflows to exactly 0.0

_NC_CACHE = {}


def build_nc():
    import concourse.mybir as mybir
    import concourse.tile as tile
    from concourse import bacc
    from concourse.masks import make_identity

    f32 = mybir.dt.float32
    f32r = mybir.dt.float32r
    bf16 = mybir.dt.bfloat16
    AF = mybir.ActivationFunctionType
    ALU = mybir.AluOpType

    nc = bacc.Bacc("TRN2", target_bir_lowering=False, debug=False,
                   num_devices=NCORES)
    # xt[kk, g, kc, ti*128+m] = x[row g*512+ti*128+m, kc*128+kk] (host-
    # pretransposed, bf16): DMAs straight into the matmul-ready layout.
    xt = nc.dram_tensor("xt", [P, NGRP * 16 * NTT * P], bf16,
                        kind="ExternalInput")
    # Host-relayouted weights: chunk-major so every DMA chunk is contiguous
    # in DRAM (4KB+ per-partition runs).
    # wqkv[cb, p, ko*128+c'] = w_qkv[ko*128+p, cb*128+c']
    wqkv = nc.dram_tensor("wqkv", [48, P, 16 * P], bf16,
                          kind="ExternalInput")
    # wout[nch, p, co*512+n'] = w_out[co*128+p, nch*512+n']
    wout = nc.dram_tensor("wout", [4, P, 16 * 512], bf16,
                          kind="ExternalInput")
    out = nc.dram_tensor("out", [ROWS, E], f32, kind="ExternalOutput")

    xt_v = xt.ap().rearrange("p (g kc tm) -> p g kc tm", g=NGRP, kc=16)

    with tile.TileContext(nc) as tc:
        with (
            tc.tile_pool(name="const", bufs=1) as cpool,
            tc.tile_pool(name="atp", bufs=2) as atpool,
            tc.tile_pool(name="qk", bufs=1) as qkpool,
            tc.tile_pool(name="ot", bufs=4) as otpool,
            tc.tile_pool(name="wq", bufs=3) as wqpool,
            tc.tile_pool(name="wo", bufs=4) as wopool,
            tc.tile_pool(name="attw", bufs=5) as awpool,
            tc.tile_pool(name="recp", bufs=2) as rpool,
            tc.tile_pool(name="vn", bufs=4) as vnpool,
            tc.tile_pool(name="osb", bufs=2) as ospool,
            tc.tile_pool(name="psQ", bufs=2, space="PSUM") as psQ,
            tc.tile_pool(name="ps2", bufs=2, space="PSUM") as ps2,
            tc.tile_pool(name="psO", bufs=1, space="PSUM") as psO,
        ):
            ident = cpool.tile([P, P], bf16, tag="ident")
            make_identity(nc, ident[:])
            # tri[kk, n] = 0 iff n >= kk else NEG: the within-tile causal
            # boundary (q-local offset n vs k-partition kk).
            tri = cpool.tile([P, P], f32, tag="tri")
            nc.gpsimd.memset(tri[:], 0.0)
            nc.gpsimd.affine_select(
                out=tri[:], in_=tri[:],
                compare_op=ALU.is_ge, fill=NEG,
                base=0, channel_multiplier=-1, pattern=[[1, P]],
            )
            # all-ones stationary (f32, matmul'd as f32r): den matmul
            # out[m,n] = sum_k ptsum[k,n] broadcast to all 128 partitions.
            ones = cpool.tile([P, P], bf16, tag="ones")
            nc.gpsimd.memset(ones[:], 1.0)

            at_g = [atpool.tile([P, 16, NTT * P], bf16, tag="at_all",
                                name=f"at{g}") for g in range(NGRP)]
            for kq in range(4):
                nc.sync.dma_start(at_g[0][:, kq * 4:(kq + 1) * 4, :],
                                  xt_v[:, 0, kq * 4:(kq + 1) * 4, :])
            wos = [wopool.tile([P, 16, 512], bf16, tag="wo",
                               name=f"wo{nch}") for nch in range(4)]

            # Output projection for one (task, nch) pair: a dense 16-matmul
            # chain, injected into later tasks' attention to fill exp-latency
            # bubbles on the PE.
            pending_fill = []

            def emit_oproj_chain(row, ot_t, nch):
                lt = ot_t.rearrange("d qt (i j) -> d qt i j", j=16)
                ps = psQ.tile([P, 512], f32, tag="mm512")
                for cc in range(16):
                    nc.tensor.matmul(
                        ps[:], lt[:, :, :, cc], wos[nch][:, cc, :],
                        start=(cc == 0), stop=(cc == 15))
                osb = ospool.tile([P, 512], f32, tag="osb")
                nc.vector.tensor_copy(osb[:], ps[:])
                nc.scalar.dma_start(
                    out.ap()[row * P:(row + 1) * P,
                             nch * 512:(nch + 1) * 512], osb[:])

            def inject_oproj():
                if pending_fill:
                    pending_fill.pop(0)()

            for g in range(NGRP):
                qt_all = qkpool.tile([P, NTT, S], bf16, tag="qtc")
                kt_all = qkpool.tile([P, NTT, S], bf16, tag="ktc")
                vt_all = qkpool.tile([P, NTT, S], bf16, tag="vtc")
                dsts = (qt_all, kt_all, vt_all)

                # ---------------- QKV phase ----------------
                # v columns first so task 0's vnat transposes (the first PE
                # work of the attention phase) never wait on the tail copies.
                for cb in list(range(32, 48)) + list(range(16, 32)) + \
                        list(range(16)):
                    wq = wqpool.tile([P, 16, P], bf16, tag="wq")
                    nc.scalar.dma_start(
                        wq[:].rearrange("p a b -> p (a b)"), wqkv.ap()[cb])
                    ps = psQ.tile([P, NTT * P], f32, tag="mm512")
                    for kc in range(16):
                        nc.tensor.matmul(
                            ps[:], wq[:, kc, :], at_g[g][:, kc, :],
                            start=(kc == 0), stop=(kc == 15))
                    j = cb % 16
                    nc.vector.tensor_copy(
                        dsts[cb // 16].rearrange(
                            "d t (i j) -> d t i j", j=16)[:, :, :, j],
                        ps[:].rearrange("d (t m) -> d t m", t=NTT))
                    if cb % 8 == 7:
                        inject_oproj()

                # ---------------- attention phase (per task) ----------------
                if g == 0:
                    for nch in range(4):
                        nc.sync.dma_start(
                            wos[nch][:].rearrange("p a b -> p (a b)"),
                            wout.ap()[nch])
                if g + 1 < NGRP:
                    for kq in range(4):
                        nc.sync.dma_start(
                            at_g[g + 1][:, kq * 4:(kq + 1) * 4, :],
                            xt_v[:, g + 1, kq * 4:(kq + 1) * 4, :])
                # V natural tiles: vnats[ti][kk, kt, d] = V[kt*128+kk, d].
                # Task 0's is built inline; later tasks' halves go through the
                # fill queue so they execute inside earlier tasks' exp bubbles.
                vnats = [vnpool.tile([P, 16, P], bf16, tag="vnat",
                                     name=f"vn{ti}") for ti in range(NTT)]

                def emit_vnat_half(ti, half):
                    tp = psQ.tile([P, 8 * P], bf16, tag="mm512")
                    for sb in range(8):
                        kt = half * 8 + sb
                        nc.tensor.transpose(
                            tp[:, sb * P:(sb + 1) * P],
                            vt_all[:, ti, kt * P:(kt + 1) * P],
                            ident[:])
                    nc.vector.tensor_copy(
                        vnats[ti][:, half * 8:(half + 1) * 8, :].rearrange(
                            "p s d -> p (s d)").bitcast(f32),
                        tp[:].bitcast(f32))

                emit_vnat_half(0, 0)
                emit_vnat_half(0, 1)
                for ti in (1, 2):
                    for half in range(2):
                        pending_fill.append(
                            lambda ti=ti, half=half: emit_vnat_half(ti, half))

                ots = []
                for ti in range(NTT):
                    vnat = vnats[ti]
                    ot = otpool.tile([P, 16, P], bf16, tag="ot")  # O^T
                    ots.append(ot)
                    for qc in range(4):
                        # Generations: (kt, width, s2 col, q offset) entries
                        # packed into one [128,<=1024] PSUM tile + one exp.
                        # Full k-tiles in pairs; causal k-tiles kt=4qc+r only
                        # cover live q cols [r*128, 512).
                        gens = []
                        for gp in range(2 * qc):
                            gens.append([(2 * gp, 512, 0, 0, False),
                                         (2 * gp + 1, 512, 512, 0, False)])
                        gens.append([(4 * qc, 512, 0, 0, True),
                                     (4 * qc + 1, 384, 512, 128, True)])
                        gens.append([(4 * qc + 2, 256, 0, 256, True),
                                     (4 * qc + 3, 128, 256, 384, True)])
                        nge = len(gens)

                        ot_ps = psO.tile([P, 512], f32, tag="otacc")
                        den_ps = psO.tile([P, 512], f32, tag="denacc")

                        pts = [None] * nge

                        def emit_qk(gi):
                            s2 = ps2.tile([P, 1024], f32, tag="s2")
                            totw = 0
                            for (kt, w, c0, q0, dg) in gens[gi]:
                                nc.tensor.matmul(
                                    s2[:, c0:c0 + w],
                                    kt_all[:, ti, kt * P:(kt + 1) * P],
                                    qt_all[:, ti,
                                           qc * 512 + q0:qc * 512 + q0 + w],
                                    start=True, stop=True)
                                totw = c0 + w
                            for (kt, w, c0, q0, dg) in gens[gi]:
                                if dg:
                                    nc.vector.tensor_tensor(
                                        s2[:, c0:c0 + P], s2[:, c0:c0 + P],
                                        tri[:], ALU.add)
                            pt = awpool.tile([P, 1024], bf16, tag="pt")
                            nc.scalar.activation(
                                pt[:, :totw], s2[:, :totw], AF.Exp,
                                bias=1.0, scale=SCALE)
                            pts[gi] = pt

                        def emit_pv(gi):
                            for ei, (kt, w, c0, q0, dg) in enumerate(gens[gi]):
                                first = gi == 0 and ei == 0
                                last = gi == nge - 1 and ei == 1
                                nc.tensor.matmul(
                                    ot_ps[:, q0:512],
                                    vnat[:, kt, :], pts[gi][:, c0:c0 + w],
                                    start=first, stop=last,
                                    skip_group_check=True)
                                nc.tensor.matmul(
                                    den_ps[:, q0:512],
                                    ones[:], pts[gi][:, c0:c0 + w],
                                    start=first, stop=last,
                                    skip_group_check=True)

                        emit_qk(0)
                        if nge > 1:
                            emit_qk(1)
                        inject_oproj()
                        for gi in range(nge):
                            emit_pv(gi)
                            if gi + 2 < nge:
                                emit_qk(gi + 2)
                        rec = rpool.tile([P, 512], f32, tag="rec")
                        nc.vector.reciprocal_approx_fast(
                            out=rec[:], in_=den_ps[:])
                        nc.vector.tensor_tensor(
                            ot[:, qc * 4:(qc + 1) * 4, :].rearrange(
                                "p s d -> p (s d)"),
                            ot_ps[:], rec[:], ALU.mult)

                    if ti == 0:
                        for half in range(2):
                            pending_fill.append(
                                lambda half=half: emit_vnat_half(3, half))
                    for nch in range(4):
                        pending_fill.append(
                            lambda row=g * NTT + ti, ot_t=ot, nch=nch:
                            emit_oproj_chain(row, ot_t, nch))

            while pending_fill:
                inject_oproj()
    nc.compile()
    return nc


def get_nc():
    if "nc" not in _NC_CACHE:
        _NC_CACHE["nc"] = build_nc()
    return _NC_CACHE["nc"]


def make_in_maps(x, w_qkv, w_out):
    import ml_dtypes

    bf = ml_dtypes.bfloat16
    xf = np.ascontiguousarray(np.asarray(x, dtype=np.float32)).reshape(
        B * S, E).astype(bf)
    wqkv_b = np.ascontiguousarray(
        np.asarray(w_qkv, dtype=np.float32).astype(bf).reshape(
            16, P, 48, P).transpose(2, 1, 0, 3).reshape(48, P, 16 * P))
    wout_b = np.ascontiguousarray(
        np.asarray(w_out, dtype=np.float32).astype(bf).reshape(
            16, P, 4, 512).transpose(2, 1, 0, 3).reshape(4, P, 16 * 512))
    # xt[c][kk, g, kc, ti, m] = x[c*1024 + g*512 + ti*128 + m, kc*128 + kk]
    xa = xf.reshape(NCORES, NGRP, NTT, P, 16, P).transpose(0, 5, 1, 4, 2, 3)
    in_maps = [
        {"xt": np.ascontiguousarray(xa[c]).reshape(P, NGRP * 16 * NTT * P),
         "wqkv": wqkv_b, "wout": wout_b}
        for c in range(NCORES)
    ]
    return in_maps


def kernel(x, w_qkv, w_out):
    from concourse.bass_utils import run_bass_kernel_spmd

    nc = get_nc()
    in_maps = make_in_maps(x, w_qkv, w_out)
    res = run_bass_kernel_spmd(nc, in_maps, core_ids=list(range(NCORES)))
    outs = [res.results[c]["out"] for c in range(NCORES)]
    return np.concatenate(outs, axis=0).reshape(B, S, E).astype(np.float32)


# revision 27
# speedup vs baseline: 1.0053x; 1.0053x over previous
"""Causal attention kernel for trn2, sharded over 8 NeuronCores.

Problem (B=4, S=2048, E=2048, H=16, D=128), fp32 in/out:
    qkv = x @ w_qkv; q,k,v = split(qkv)
    q,k,v reshaped (B,S,E)->(B,H,S,D) as a RAW view (no transpose), i.e.
    per (b,h): Q_h = rows [h*128,(h+1)*128) of q[b] reinterpreted [S,D].
    o = softmax(QK^T/sqrt(D) + causal(+1/-10000)) @ V, inverse raw view,
    out = o @ w_out.

The raw view maps head h to a contiguous block of 128 sequence rows, so
the computation splits into B*H = 64 independent tasks; core c gets 8
tasks = rows [c*1024,(c+1)*1024) of x.reshape(B*S, E).  No collectives.

All matmuls run in bf16 (inputs converted, x pre-transposed, and the
weights re-laid-out chunk-major host-side so every DMA chunk is DRAM-
contiguous); accumulation is fp32 in PSUM.  Attention computes
S^T = K Q^T per (q-chunk 512, k-tile 128) with causal diagonal tiles cut
to their live q-range, exp batched per [128,<=1024] PSUM generation, a
softmax denominator via ones-stationary matmuls (partition-dim sum
broadcast), and a software-pipelined generation loop (QK lookahead 2).
Each task's output-projection chains and the next tasks' V-transposes
are deferred into a fill queue and injected at later tasks' q-chunk
starts, filling the exp-latency bubbles on the PE; leftovers drain into
the next group's QKV stream.
"""

import numpy as np

B, S, E = 4, 2048, 2048
H, D, P = 16, 128, 128
NCORES = 8
ROWS = B * S // NCORES   # 1024 rows per core = 8 tasks of 128 rows
NGRP = 2                 # task groups per core
NTT = 4                  # tasks per group
SCALE = float(1.0 / np.sqrt(D))
NEG = -1.0e9  # pre-scale additive mask; exp underflows to exactly 0.0

_NC_CACHE = {}


def build_nc():
    import concourse.mybir as mybir
    import concourse.tile as tile
    from concourse import bacc
    from concourse.masks import make_identity

    f32 = mybir.dt.float32
    f32r = mybir.dt.float32r
    bf16 = mybir.dt.bfloat16
    AF = mybir.ActivationFunctionType
    ALU = mybir.AluOpType

    nc = bacc.Bacc("TRN2", target_bir_lowering=False, debug=False,
                   num_devices=NCORES)
    # xt[kk, g, kc, ti*128+m] = x[row g*512+ti*128+m, kc*128+kk] (host-
    # pretransposed, bf16): DMAs straight into the matmul-ready layout.
    xt = nc.dram_tensor("xt", [P, NGRP * 16 * NTT * P], bf16,
                        kind="ExternalInput")
    # Host-relayouted weights: chunk-major so every DMA chunk is contiguous
    # in DRAM (4KB+ per-partition runs).
    # wqkv[cb, p, ko*128+c'] = w_qkv[ko*128+p, cb*128+c']
    wqkv = nc.dram_tensor("wqkv", [48, P, 16 * P], bf16,
                          kind="ExternalInput")
    # wout[nch, p, co*512+n'] = w_out[co*128+p, nch*512+n']
    wout = nc.dram_tensor("wout", [4, P, 16 * 512], bf16,
                          kind="ExternalInput")
    out = nc.dram_tensor("out", [ROWS, E], f32, kind="ExternalOutput")

    xt_v = xt.ap().rearrange("p (g kc tm) -> p g kc tm", g=NGRP, kc=16)

    with tile.TileContext(nc) as tc:
        with (
            tc.tile_pool(name="const", bufs=1) as cpool,
            tc.tile_pool(name="atp", bufs=2) as atpool,
            tc.tile_pool(name="qk", bufs=1) as qkpool,
            tc.tile_pool(name="ot", bufs=4) as otpool,
            tc.tile_pool(name="wq", bufs=3) as wqpool,
            tc.tile_pool(name="wo", bufs=4) as wopool,
            tc.tile_pool(name="attw", bufs=5) as awpool,
            tc.tile_pool(name="recp", bufs=2) as rpool,
            tc.tile_pool(name="vn", bufs=4) as vnpool,
            tc.tile_pool(name="osb", bufs=2) as ospool,
            tc.tile_pool(name="psQ", bufs=2, space="PSUM") as psQ,
            tc.tile_pool(name="ps2", bufs=2, space="PSUM") as ps2,
            tc.tile_pool(name="psO", bufs=1, space="PSUM") as psO,
        ):
            ident = cpool.tile([P, P], bf16, tag="ident")
            make_identity(nc, ident[:])
            # tri[kk, n] = 0 iff n >= kk else NEG: the within-tile causal
            # boundary (q-local offset n vs k-partition kk).
            tri = cpool.tile([P, P], f32, tag="tri")
            nc.gpsimd.memset(tri[:], 0.0)
            nc.gpsimd.affine_select(
                out=tri[:], in_=tri[:],
                compare_op=ALU.is_ge, fill=NEG,
                base=0, channel_multiplier=-1, pattern=[[1, P]],
            )
            # all-ones stationary (f32, matmul'd as f32r): den matmul
            # out[m,n] = sum_k ptsum[k,n] broadcast to all 128 partitions.
            ones = cpool.tile([P, P], bf16, tag="ones")
            nc.gpsimd.memset(ones[:], 1.0)

            at_g = [atpool.tile([P, 16, NTT * P], bf16, tag="at_all",
                                name=f"at{g}") for g in range(NGRP)]
            for kq in range(4):
                nc.sync.dma_start(at_g[0][:, kq * 4:(kq + 1) * 4, :],
                                  xt_v[:, 0, kq * 4:(kq + 1) * 4, :])
            wos = [wopool.tile([P, 16, 512], bf16, tag="wo",
                               name=f"wo{nch}") for nch in range(4)]

            # Output projection for one (task, nch) pair: a dense 16-matmul
            # chain, injected into later tasks' attention to fill exp-latency
            # bubbles on the PE.
            pending_fill = []

            def emit_oproj_chain(row, ot_t, nch):
                lt = ot_t.rearrange("d qt (i j) -> d qt i j", j=16)
                ps = psQ.tile([P, 512], f32, tag="mm512")
                for cc in range(16):
                    nc.tensor.matmul(
                        ps[:], lt[:, :, :, cc], wos[nch][:, cc, :],
                        start=(cc == 0), stop=(cc == 15))
                osb = ospool.tile([P, 512], f32, tag="osb")
                nc.vector.tensor_copy(osb[:], ps[:])
                nc.scalar.dma_start(
                    out.ap()[row * P:(row + 1) * P,
                             nch * 512:(nch + 1) * 512], osb[:])

            def inject_oproj():
                if pending_fill:
                    pending_fill.pop(0)()

            for g in range(NGRP):
                qt_all = qkpool.tile([P, NTT, S], bf16, tag="qtc")
                kt_all = qkpool.tile([P, NTT, S], bf16, tag="ktc")
                vt_all = qkpool.tile([P, NTT, S], bf16, tag="vtc")
                dsts = (qt_all, kt_all, vt_all)

                # ---------------- QKV phase ----------------
                for cb in range(48):
                    wq = wqpool.tile([P, 16, P], bf16, tag="wq")
                    nc.scalar.dma_start(
                        wq[:].rearrange("p a b -> p (a b)"), wqkv.ap()[cb])
                    ps = psQ.tile([P, NTT * P], f32, tag="mm512")
                    for kc in range(16):
                        nc.tensor.matmul(
                            ps[:], wq[:, kc, :], at_g[g][:, kc, :],
                            start=(kc == 0), stop=(kc == 15))
                    j = cb % 16
                    nc.vector.tensor_copy(
                        dsts[cb // 16].rearrange(
                            "d t (i j) -> d t i j", j=16)[:, :, :, j],
                        ps[:].rearrange("d (t m) -> d t m", t=NTT))
                    if cb % 8 == 7:
                        inject_oproj()

                # ---------------- attention phase (per task) ----------------
                if g == 0:
                    for nch in range(4):
                        nc.sync.dma_start(
                            wos[nch][:].rearrange("p a b -> p (a b)"),
                            wout.ap()[nch])
                if g + 1 < NGRP:
                    for kq in range(4):
                        nc.sync.dma_start(
                            at_g[g + 1][:, kq * 4:(kq + 1) * 4, :],
                            xt_v[:, g + 1, kq * 4:(kq + 1) * 4, :])
                # V natural tiles: vnats[ti][kk, kt, d] = V[kt*128+kk, d].
                # Task 0's is built inline; later tasks' halves go through the
                # fill queue so they execute inside earlier tasks' exp bubbles.
                vnats = [vnpool.tile([P, 16, P], bf16, tag="vnat",
                                     name=f"vn{ti}") for ti in range(NTT)]

                def emit_vnat_half(ti, half):
                    tp = psQ.tile([P, 8 * P], bf16, tag="mm512")
                    for sb in range(8):
                        kt = half * 8 + sb
                        nc.tensor.transpose(
                            tp[:, sb * P:(sb + 1) * P],
                            vt_all[:, ti, kt * P:(kt + 1) * P],
                            ident[:])
                    nc.vector.tensor_copy(
                        vnats[ti][:, half * 8:(half + 1) * 8, :].rearrange(
                            "p s d -> p (s d)").bitcast(f32),
                        tp[:].bitcast(f32))

                emit_vnat_half(0, 0)
                emit_vnat_half(0, 1)
                for ti in (1, 2):
                    for half in range(2):
                        pending_fill.append(
                            lambda ti=ti, half=half: emit_vnat_half(ti, half))

                ots = []
                for ti in range(NTT):
                    vnat = vnats[ti]
                    ot = otpool.tile([P, 16, P], bf16, tag="ot")  # O^T
                    ots.append(ot)
                    for qc in range(4):
                        # Generations: (kt, width, s2 col, q offset) entries
                        # packed into one [128,<=1024] PSUM tile + one exp.
                        # Full k-tiles in pairs; causal k-tiles kt=4qc+r only
                        # cover live q cols [r*128, 512).
                        gens = []
                        for gp in range(2 * qc):
                            gens.append([(2 * gp, 512, 0, 0, False),
                                         (2 * gp + 1, 512, 512, 0, False)])
                        gens.append([(4 * qc, 512, 0, 0, True),
                                     (4 * qc + 1, 384, 512, 128, True)])
                        gens.append([(4 * qc + 2, 256, 0, 256, True),
                                     (4 * qc + 3, 128, 256, 384, True)])
                        nge = len(gens)

                        ot_ps = psO.tile([P, 512], f32, tag="otacc")
                        den_ps = psO.tile([P, 512], f32, tag="denacc")

                        pts = [None] * nge

                        def emit_qk(gi):
                            s2 = ps2.tile([P, 1024], f32, tag="s2")
                            totw = 0
                            for (kt, w, c0, q0, dg) in gens[gi]:
                                nc.tensor.matmul(
                                    s2[:, c0:c0 + w],
                                    kt_all[:, ti, kt * P:(kt + 1) * P],
                                    qt_all[:, ti,
                                           qc * 512 + q0:qc * 512 + q0 + w],
                                    start=True, stop=True)
                                totw = c0 + w
                            for (kt, w, c0, q0, dg) in gens[gi]:
                                if dg:
                                    nc.vector.tensor_tensor(
                                        s2[:, c0:c0 + P], s2[:, c0:c0 + P],
                                        tri[:], ALU.add)
                            pt = awpool.tile([P, 1024], bf16, tag="pt")
                            nc.scalar.activation(
                                pt[:, :totw], s2[:, :totw], AF.Exp,
                                bias=1.0, scale=SCALE)
                            pts[gi] = pt

                        def emit_pv(gi):
                            for ei, (kt, w, c0, q0, dg) in enumerate(gens[gi]):
                                first = gi == 0 and ei == 0
                                last = gi == nge - 1 and ei == 1
                                nc.tensor.matmul(
                                    ot_ps[:, q0:512],
                                    vnat[:, kt, :], pts[gi][:, c0:c0 + w],
                                    start=first, stop=last,
                                    skip_group_check=True)
                                nc.tensor.matmul(
                                    den_ps[:, q0:512],
                                    ones[:], pts[gi][:, c0:c0 + w],
                                    start=first, stop=last,
                                    skip_group_check=True)

                        emit_qk(0)
                        if nge > 1:
                            emit_qk(1)
                        inject_oproj()
                        for gi in range(nge):
                            emit_pv(gi)
                            if gi + 2 < nge:
                                emit_qk(gi + 2)
                        rec = rpool.tile([P, 512], f32, tag="rec")
                        nc.vector.reciprocal_approx_fast(
                            out=rec[:], in_=den_ps[:])
                        nc.vector.tensor_tensor(
                            ot[:, qc * 4:(qc + 1) * 4, :].rearrange(
                                "p s d -> p (s d)"),
                            ot_ps[:], rec[:], ALU.mult)

                    if ti == 0:
                        for half in range(2):
                            pending_fill.append(
                                lambda half=half: emit_vnat_half(3, half))
                    for nch in range(4):
                        pending_fill.append(
                            lambda row=g * NTT + ti, ot_t=ot, nch=nch:
                            emit_oproj_chain(row, ot_t, nch))

            while pending_fill:
                inject_oproj()
    nc.compile()
    return nc


def get_nc():
    if "nc" not in _NC_CACHE:
        _NC_CACHE["nc"] = build_nc()
    return _NC_CACHE["nc"]


def make_in_maps(x, w_qkv, w_out):
    import ml_dtypes

    bf = ml_dtypes.bfloat16
    xf = np.ascontiguousarray(np.asarray(x, dtype=np.float32)).reshape(
        B * S, E).astype(bf)
    wqkv_b = np.ascontiguousarray(
        np.asarray(w_qkv, dtype=np.float32).astype(bf).reshape(
            16, P, 48, P).transpose(2, 1, 0, 3).reshape(48, P, 16 * P))
    wout_b = np.ascontiguousarray(
        np.asarray(w_out, dtype=np.float32).astype(bf).reshape(
            16, P, 4, 512).transpose(2, 1, 0, 3).reshape(4, P, 16 * 512))
    # xt[c][kk, g, kc, ti, m] = x[c*1024 + g*512 + ti*128 + m, kc*128 + kk]
    xa = xf.reshape(NCORES, NGRP, NTT, P, 16, P).transpose(0, 5, 1, 4, 2, 3)
    in_maps = [
        {"xt": np.ascontiguousarray(xa[c]).reshape(P, NGRP * 16 * NTT * P),
         "wqkv": wqkv_b, "wout": wout_b}
        for c in range(NCORES)
    ]
    return in_maps


def kernel(x, w_qkv, w_out):
    from concourse.bass_utils import run_bass_kernel_spmd

    nc = get_nc()
    in_maps = make_in_maps(x, w_qkv, w_out)
    res = run_bass_kernel_spmd(nc, in_maps, core_ids=list(range(NCORES)))
    outs = [res.results[c]["out"] for c in range(NCORES)]
    return np.concatenate(outs, axis=0).reshape(B, S, E).astype(np.float32)


# revision 29
# speedup vs baseline: 1.0328x; 1.0274x over previous
"""Causal attention kernel for trn2, sharded over 8 NeuronCores.

Problem (B=4, S=2048, E=2048, H=16, D=128), fp32 in/out:
    qkv = x @ w_qkv; q,k,v = split(qkv)
    q,k,v reshaped (B,S,E)->(B,H,S,D) as a RAW view (no transpose), i.e.
    per (b,h): Q_h = rows [h*128,(h+1)*128) of q[b] reinterpreted [S,D].
    o = softmax(QK^T/sqrt(D) + causal(+1/-10000)) @ V, inverse raw view,
    out = o @ w_out.

The raw view maps head h to a contiguous block of 128 sequence rows, so
the computation splits into B*H = 64 independent tasks; core c gets 8
tasks = rows [c*1024,(c+1)*1024) of x.reshape(B*S, E).  No collectives.

All matmuls run in bf16 (inputs converted, x pre-transposed, and the
weights re-laid-out chunk-major host-side so every DMA chunk is DRAM-
contiguous); accumulation is fp32 in PSUM.  Attention computes
S^T = K Q^T per (q-chunk 512, k-tile 128) with causal diagonal tiles cut
to their live q-range, exp batched per [128,<=1024] PSUM generation, a
softmax denominator via ones-stationary matmuls (partition-dim sum
broadcast), and a software-pipelined generation loop (QK lookahead 2).
Each task's output-projection chains and the next tasks' V-transposes
are deferred into a fill queue and injected at later tasks' q-chunk
starts, filling the exp-latency bubbles on the PE; leftovers drain into
the next group's QKV stream.
"""

import numpy as np

B, S, E = 4, 2048, 2048
H, D, P = 16, 128, 128
NCORES = 8
ROWS = B * S // NCORES   # 1024 rows per core = 8 tasks of 128 rows
NGRP = 2                 # task groups per core
NTT = 4                  # tasks per group
SCALE = float(1.0 / np.sqrt(D))
NEG = -1.0e9  # pre-scale additive mask; exp underflows to exactly 0.0

_NC_CACHE = {}


def build_nc():
    import concourse.mybir as mybir
    import concourse.tile as tile
    from concourse import bacc
    from concourse.masks import make_identity

    f32 = mybir.dt.float32
    f32r = mybir.dt.float32r
    bf16 = mybir.dt.bfloat16
    AF = mybir.ActivationFunctionType
    ALU = mybir.AluOpType

    nc = bacc.Bacc("TRN2", target_bir_lowering=False, debug=False,
                   num_devices=NCORES)
    # xt[kk, g, kc, ti*128+m] = x[row g*512+ti*128+m, kc*128+kk] (host-
    # pretransposed, bf16): DMAs straight into the matmul-ready layout.
    xt = nc.dram_tensor("xt", [P, NGRP * 16 * NTT * P], bf16,
                        kind="ExternalInput")
    # Host-relayouted weights: chunk-major so every DMA chunk is contiguous
    # in DRAM (4KB+ per-partition runs).
    # wqkv[cb, p, ko*128+c'] = w_qkv[ko*128+p, cb*128+c']
    wqkv = nc.dram_tensor("wqkv", [48, P, 16 * P], bf16,
                          kind="ExternalInput")
    # wout[nch, p, co*512+n'] = w_out[co*128+p, nch*512+n']
    wout = nc.dram_tensor("wout", [4, P, 16 * 512], bf16,
                          kind="ExternalInput")
    out = nc.dram_tensor("out", [ROWS, E], f32, kind="ExternalOutput")

    xt_v = xt.ap().rearrange("p (g kc tm) -> p g kc tm", g=NGRP, kc=16)

    with tile.TileContext(nc) as tc:
        with (
            tc.tile_pool(name="const", bufs=1) as cpool,
            tc.tile_pool(name="atp", bufs=2) as atpool,
            tc.tile_pool(name="qk", bufs=1) as qkpool,
            tc.tile_pool(name="ot", bufs=4) as otpool,
            tc.tile_pool(name="wq", bufs=3) as wqpool,
            tc.tile_pool(name="wo", bufs=4) as wopool,
            tc.tile_pool(name="attw", bufs=6) as awpool,
            tc.tile_pool(name="recp", bufs=2) as rpool,
            tc.tile_pool(name="vn", bufs=4) as vnpool,
            tc.tile_pool(name="osb", bufs=2) as ospool,
            tc.tile_pool(name="psQ", bufs=2, space="PSUM") as psQ,
            tc.tile_pool(name="ps2", bufs=4, space="PSUM") as ps2,
            tc.tile_pool(name="psO", bufs=1, space="PSUM") as psO,
        ):
            ident = cpool.tile([P, P], bf16, tag="ident")
            make_identity(nc, ident[:])
            # tri[kk, n] = 0 iff n >= kk else NEG: the within-tile causal
            # boundary (q-local offset n vs k-partition kk).
            tri = cpool.tile([P, P], f32, tag="tri")
            nc.gpsimd.memset(tri[:], 0.0)
            nc.gpsimd.affine_select(
                out=tri[:], in_=tri[:],
                compare_op=ALU.is_ge, fill=NEG,
                base=0, channel_multiplier=-1, pattern=[[1, P]],
            )
            # all-ones stationary (f32, matmul'd as f32r): den matmul
            # out[m,n] = sum_k ptsum[k,n] broadcast to all 128 partitions.
            ones = cpool.tile([P, P], bf16, tag="ones")
            nc.gpsimd.memset(ones[:], 1.0)

            at_g = [atpool.tile([P, 16, NTT * P], bf16, tag="at_all",
                                name=f"at{g}") for g in range(NGRP)]
            for kq in range(4):
                nc.sync.dma_start(at_g[0][:, kq * 4:(kq + 1) * 4, :],
                                  xt_v[:, 0, kq * 4:(kq + 1) * 4, :])
            wos = [wopool.tile([P, 16, 512], bf16, tag="wo",
                               name=f"wo{nch}") for nch in range(4)]

            # Output projection for one (task, nch) pair: a dense 16-matmul
            # chain, injected into later tasks' attention to fill exp-latency
            # bubbles on the PE.
            pending_fill = []

            def emit_oproj_chain(row, ot_t, nch):
                lt = ot_t.rearrange("d qt (i j) -> d qt i j", j=16)
                ps = psQ.tile([P, 512], f32, tag="mm512")
                for cc in range(16):
                    nc.tensor.matmul(
                        ps[:], lt[:, :, :, cc], wos[nch][:, cc, :],
                        start=(cc == 0), stop=(cc == 15))
                osb = ospool.tile([P, 512], f32, tag="osb")
                nc.vector.tensor_copy(osb[:], ps[:])
                nc.scalar.dma_start(
                    out.ap()[row * P:(row + 1) * P,
                             nch * 512:(nch + 1) * 512], osb[:])

            def inject_oproj():
                if pending_fill:
                    pending_fill.pop(0)()

            for g in range(NGRP):
                qt_all = qkpool.tile([P, NTT, S], bf16, tag="qtc")
                kt_all = qkpool.tile([P, NTT, S], bf16, tag="ktc")
                vt_all = qkpool.tile([P, NTT, S], bf16, tag="vtc")
                dsts = (qt_all, kt_all, vt_all)

                # ---------------- QKV phase ----------------
                # Order q, v, k: the attention phase's first PE work (task
                # 0's vnat transposes, ~2.6us) depends on the v copies
                # (mid-stream), covering the wait for the tail k copies.
                for cb in list(range(16)) + list(range(32, 48)) + \
                        list(range(16, 32)):
                    wq = wqpool.tile([P, 16, P], bf16, tag="wq")
                    nc.scalar.dma_start(
                        wq[:].rearrange("p a b -> p (a b)"), wqkv.ap()[cb])
                    ps = psQ.tile([P, NTT * P], f32, tag="mm512")
                    for kc in range(16):
                        nc.tensor.matmul(
                            ps[:], wq[:, kc, :], at_g[g][:, kc, :],
                            start=(kc == 0), stop=(kc == 15))
                    j = cb % 16
                    nc.vector.tensor_copy(
                        dsts[cb // 16].rearrange(
                            "d t (i j) -> d t i j", j=16)[:, :, :, j],
                        ps[:].rearrange("d (t m) -> d t m", t=NTT))
                    if cb % 8 == 7:
                        inject_oproj()

                # ---------------- attention phase (per task) ----------------
                if g == 0:
                    for nch in range(4):
                        nc.sync.dma_start(
                            wos[nch][:].rearrange("p a b -> p (a b)"),
                            wout.ap()[nch])
                if g + 1 < NGRP:
                    for kq in range(4):
                        nc.sync.dma_start(
                            at_g[g + 1][:, kq * 4:(kq + 1) * 4, :],
                            xt_v[:, g + 1, kq * 4:(kq + 1) * 4, :])
                # V natural tiles: vnats[ti][kk, kt, d] = V[kt*128+kk, d].
                # Task 0's is built inline; later tasks' halves go through the
                # fill queue so they execute inside earlier tasks' exp bubbles.
                vnats = [vnpool.tile([P, 16, P], bf16, tag="vnat",
                                     name=f"vn{ti}") for ti in range(NTT)]

                def emit_vnat_half(ti, half):
                    tp = psQ.tile([P, 8 * P], bf16, tag="mm512")
                    for sb in range(8):
                        kt = half * 8 + sb
                        nc.tensor.transpose(
                            tp[:, sb * P:(sb + 1) * P],
                            vt_all[:, ti, kt * P:(kt + 1) * P],
                            ident[:])
                    nc.vector.tensor_copy(
                        vnats[ti][:, half * 8:(half + 1) * 8, :].rearrange(
                            "p s d -> p (s d)").bitcast(f32),
                        tp[:].bitcast(f32))

                emit_vnat_half(0, 0)
                emit_vnat_half(0, 1)
                for ti in (1, 2):
                    for half in range(2):
                        pending_fill.append(
                            lambda ti=ti, half=half: emit_vnat_half(ti, half))

                ots = []
                for ti in range(NTT):
                    vnat = vnats[ti]
                    ot = otpool.tile([P, 16, P], bf16, tag="ot")  # O^T
                    ots.append(ot)
                    for qc in range(4):
                        # Generations: (kt, width, s2 col, q offset) entries
                        # packed into one [128,<=1024] PSUM tile + one exp.
                        # Full k-tiles in pairs; causal k-tiles kt=4qc+r only
                        # cover live q cols [r*128, 512).
                        gens = [(kt, 512, 0, False) for kt in range(4 * qc)]
                        for r in range(4):
                            gens.append((4 * qc + r, (4 - r) * P, r * P, True))
                        nge = len(gens)

                        ot_ps = psO.tile([P, 512], f32, tag="otacc")
                        den_ps = psO.tile([P, 512], f32, tag="denacc")

                        pts = [None] * nge

                        def emit_qk(gi):
                            kt, w, q0, dg = gens[gi]
                            s2 = ps2.tile([P, 512], f32, tag="s2")
                            nc.tensor.matmul(
                                s2[:, :w],
                                kt_all[:, ti, kt * P:(kt + 1) * P],
                                qt_all[:, ti,
                                       qc * 512 + q0:qc * 512 + q0 + w],
                                start=True, stop=True)
                            if dg:
                                nc.vector.tensor_tensor(
                                    s2[:, :P], s2[:, :P], tri[:], ALU.add)
                            pt = awpool.tile([P, 512], bf16, tag="pt")
                            nc.scalar.activation(
                                pt[:, :w], s2[:, :w], AF.Exp,
                                bias=1.0, scale=SCALE)
                            pts[gi] = pt

                        def emit_pv(gi):
                            kt, w, q0, dg = gens[gi]
                            first = gi == 0
                            last = gi == nge - 1
                            nc.tensor.matmul(
                                ot_ps[:, q0:512],
                                vnat[:, kt, :], pts[gi][:, :w],
                                start=first, stop=last,
                                skip_group_check=True)
                            nc.tensor.matmul(
                                den_ps[:, q0:512],
                                ones[:], pts[gi][:, :w],
                                start=first, stop=last,
                                skip_group_check=True)

                        for gi in range(min(3, nge)):
                            emit_qk(gi)
                        inject_oproj()
                        for gi in range(nge):
                            emit_pv(gi)
                            if gi + 3 < nge:
                                emit_qk(gi + 3)
                        rec = rpool.tile([P, 512], f32, tag="rec")
                        nc.vector.reciprocal_approx_fast(
                            out=rec[:], in_=den_ps[:])
                        nc.vector.tensor_tensor(
                            ot[:, qc * 4:(qc + 1) * 4, :].rearrange(
                                "p s d -> p (s d)"),
                            ot_ps[:], rec[:], ALU.mult)

                    if ti == 0:
                        for half in range(2):
                            pending_fill.append(
                                lambda half=half: emit_vnat_half(3, half))
                    for nch in range(4):
                        pending_fill.append(
                            lambda row=g * NTT + ti, ot_t=ot, nch=nch:
                            emit_oproj_chain(row, ot_t, nch))

            while pending_fill:
                inject_oproj()
    nc.compile()
    return nc


def get_nc():
    if "nc" not in _NC_CACHE:
        _NC_CACHE["nc"] = build_nc()
    return _NC_CACHE["nc"]


def make_in_maps(x, w_qkv, w_out):
    import ml_dtypes

    bf = ml_dtypes.bfloat16
    xf = np.ascontiguousarray(np.asarray(x, dtype=np.float32)).reshape(
        B * S, E).astype(bf)
    wqkv_b = np.ascontiguousarray(
        np.asarray(w_qkv, dtype=np.float32).astype(bf).reshape(
            16, P, 48, P).transpose(2, 1, 0, 3).reshape(48, P, 16 * P))
    wout_b = np.ascontiguousarray(
        np.asarray(w_out, dtype=np.float32).astype(bf).reshape(
            16, P, 4, 512).transpose(2, 1, 0, 3).reshape(4, P, 16 * 512))
    # xt[c][kk, g, kc, ti, m] = x[c*1024 + g*512 + ti*128 + m, kc*128 + kk]
    xa = xf.reshape(NCORES, NGRP, NTT, P, 16, P).transpose(0, 5, 1, 4, 2, 3)
    in_maps = [
        {"xt": np.ascontiguousarray(xa[c]).reshape(P, NGRP * 16 * NTT * P),
         "wqkv": wqkv_b, "wout": wout_b}
        for c in range(NCORES)
    ]
    return in_maps


def kernel(x, w_qkv, w_out):
    from concourse.bass_utils import run_bass_kernel_spmd

    nc = get_nc()
    in_maps = make_in_maps(x, w_qkv, w_out)
    res = run_bass_kernel_spmd(nc, in_maps, core_ids=list(range(NCORES)))
    outs = [res.results[c]["out"] for c in range(NCORES)]
    return np.concatenate(outs, axis=0).reshape(B, S, E).astype(np.float32)
